# revision 22
# baseline (speedup 1.0000x reference)
"""Distillation-trainer loss kernel for Trainium2 (8 NeuronCores).

Computes  loss = mean((attn(q,k,v) - attn(q,ck,cv))**2)  for
q:[1,8,1024,128], k/v:[1,8,8192,128], ck/cv:[1,8,1024,128] fp32.

Sharding: one kv-head per core (h axis, 8 heads / 8 cores). Each core
computes its head's squared-error partial sums; the host adds the 8
partials and divides by the element count (the "all-reduce" of the
scalar loss).

Per-core pipeline (head h), blocks ordered c0, t0, c1, t1 where c =
compressed attention, t = teacher attention, each over a 512-wide
q-block:
  - scores: one 512-wide matmul per 128-row n-tile (kT stationary,
    qT moving, PSUM fp32).
  - exp is split across two engines to beat the ScalarE throughput
    wall (~1.2 ns/elem): ACT groups use the Exp table; DVE groups use
    a one-instruction Schraudolph exp - t = s*A + B converted to int16,
    whose bits ARE bf16(exp(s*scale)) because bf16's exponent/mantissa
    are linear in log2. Probs land in bf16 either way (loss rel-err
    ~1e-4, tolerance 2e-2).
  - PV: ex chunk [128n x 128q] stationary, moving v-tile [128, 129]
    with a ones column appended, accumulating [z' | S] per q-quarter
    in four PSUM banks across n-tiles.
  - normalize on DVE (reciprocal of S), MSE via fused
    tensor_tensor_reduce into per-partition partials.
  - k/v HBM streams (8.4 MB) are interleaved into c0/t0 so DMA hides
    under compute; kT PSUM->SBUF copy-outs run on GpSimd to unload DVE.
"""

import numpy as np

import concourse.bass as bass
import concourse.mybir as mybir
import concourse.tile as tile
from concourse import bacc
from concourse.masks import make_identity
from concourse.bass_utils import run_bass_kernel_spmd

F32 = mybir.dt.float32
BF16 = mybir.dt.bfloat16
I16 = mybir.dt.int16
AF = mybir.ActivationFunctionType
ALU = mybir.AluOpType

B, H, Q, N, NC, D = 1, 8, 1024, 8192, 1024, 128
N_CORES = 8
SCALE = 1.0 / float(np.sqrt(D))

QC = 512                   # q-block width
N_QB = Q // QC             # 2
GT = 1                     # n-tiles per scores/exp group
NT = N // 128              # 64 teacher n-tiles
NCT = NC // 128            # 8 compressed n-tiles

# Schraudolph exp constants: int16(raw_score * EXPA + EXPB) bits == bf16
# of exp(raw_score * SCALE).  EXPA folds the softmax scale and log2(e)
# into the bf16 exponent/mantissa field width (128 = 2^7 mantissa bits).
EXPC = 5.8                 # mantissa bias correction (minimizes mean err)
EXPA = float(SCALE * np.log2(np.e) * 128.0)
EXPB = float(127.0 * 128.0 - EXPC)

# which exp groups run on DVE (rest on ACT); tuned so ACT/DVE/PE balance
DVE_C0 = frozenset((1, 3, 5, 7))
DVE_C1 = frozenset((1, 3, 5, 7))
DVE_T0 = frozenset(i for i in range(64) if i % 16 in (1, 4, 7, 10, 13))  # 20 of 64
DVE_T1 = frozenset(i for i in range(64) if i % 2 == 1)      # 32 of 64
GP_KCOPY = False           # GPSIMD cannot read PSUM (walrus birverifier)
USE_TTR = False            # tensor_tensor_reduce hangs real TRN2 (sim-only)


def _emit(nc: bass.Bass, tc: tile.TileContext, qh, kh, vh, ckh, cvh, out_dram):
    ctxs = []

    def pool(**kw):
        p = tc.tile_pool(**kw)
        ctxs.append(p)
        return p.__enter__()

    pconst = pool(name="pconst", bufs=1)
    pkst = pool(name="pkst", bufs=3)
    pvst = pool(name="pvst", bufs=3)
    pex = pool(name="pex", bufs=4)
    psmall = pool(name="psmall", bufs=4)
    psc = pool(name="psc", bufs=4, space="PSUM")
    pz = pool(name="pz", bufs=1, space="PSUM")

    # ---- persistent SBUF tensors ----
    ident = pconst.tile([128, 128], BF16, tag="ident")
    make_identity(nc, ident[:])

    kT = pconst.tile([128, NT, 128], BF16, tag="kT")        # [d, t, n]
    vb = pconst.tile([128, NT, 129], BF16, tag="vb")        # [n, t, d+1]
    qT = pconst.tile([128, 8, 128], BF16, tag="qT")         # [d, qt, q]
    ckT = pconst.tile([128, NCT, 128], BF16, tag="ckT")
    cvb = pconst.tile([128, NCT, 129], BF16, tag="cvb")
    zcomp = pconst.tile([128, Q // 128, 128], F32, tag="zcomp")  # [q, qt, d]
    accq = pconst.tile([128, Q // 128], F32, tag="accq")

    nc.gpsimd.memset(vb[:, :, 128:129], 1.0)
    nc.gpsimd.memset(cvb[:, :, 128:129], 1.0)

    # Warm the ACT exp table while prep DMAs run (~2.7us table load).
    warm = psmall.tile([128, 1], F32, tag="warm")
    nc.gpsimd.memset(warm[:], 0.0)
    warm2 = psmall.tile([128, 1], F32, tag="warm2")
    nc.scalar.activation(warm2[:], warm[:], AF.Exp)

    # ---- staging helpers ----
    def stage_transposed(src, dst, n_tiles, tag):
        # src [n, 128] fp32 -> dst [128, t, n] bf16 via cast + PE transpose
        stg = pkst.tile([128, n_tiles, 128], F32, tag=tag)
        nc.sync.dma_start(out=stg[:],
                          in_=src[:, :].rearrange("(i p) d -> p i d", p=128))
        sb = pkst.tile([128, n_tiles, 128], BF16, tag=tag + "b")
        nc.vector.tensor_copy(sb[:], stg[:])
        for g in range(n_tiles // 4):
            tp = psc.tile([128, 4, 128], BF16, tag="sp")
            for j in range(4):
                nc.tensor.transpose(tp[:, j, :], sb[:, 4 * g + j, :], ident[:])
            nc.vector.tensor_copy(dst[:, 4 * g:4 * g + 4, :], tp[:])

    def k_chunk(g):
        # 512 rows of k -> kT[:, 4g:4g+4, :]
        stg = pkst.tile([128, 4, 128], F32, tag="kstg")
        ap = kh[g * 512:(g + 1) * 512, :].rearrange("(i p) d -> p i d", p=128)
        nc.sync.dma_start(out=stg[:], in_=ap)
        kb = pkst.tile([128, 4, 128], BF16, tag="kb")
        nc.vector.tensor_copy(kb[:], stg[:])
        tp = psc.tile([128, 4, 128], BF16, tag="sp")
        for j in range(4):
            nc.tensor.transpose(tp[:, j, :], kb[:, j, :], ident[:])
        eng = nc.gpsimd if GP_KCOPY else nc.vector
        eng.tensor_copy(kT[:, 4 * g:4 * g + 4, :], tp[:])

    def v_chunk(g):
        stg = pvst.tile([128, 4, 128], F32, tag="vstg")
        ap = vh[g * 512:(g + 1) * 512, :].rearrange("(i p) d -> p i d", p=128)
        nc.sync.dma_start(out=stg[:], in_=ap)
        nc.gpsimd.tensor_copy(vb[:, 4 * g:4 * g + 4, 0:128], stg[:])

    def stage_cv():
        stg = pvst.tile([128, NCT, 128], F32, tag="cvstg")
        nc.sync.dma_start(out=stg[:],
                          in_=cvh[:, :].rearrange("(i p) d -> p i d", p=128))
        nc.vector.tensor_copy(cvb[:, :, 0:128], stg[:])

    # ---- attention for one 512-wide q-block ----
    def attend(keysT, vals, n_tiles, qb, dve_set, interleave):
        """Returns (zA, zB): zA [128,3,129] holds q-quarters 0-2, zB [128,129]
        holds quarter 3 -- each column block is [z' | S].  The four PV
        accumulation chains share PSUM banks; per-element has_written bits
        keep them independent (skip_group_check)."""
        zap = [pz.tile([128, 129], F32, tag=f"z{h}", name=f"z{h}")[:]
               for h in range(4)]
        qs = qT[:, 4 * qb:4 * qb + 4, :]

        def emit_pv(ex, t0, gn):
            for j in range(gn):
                t = t0 + j
                st = dict(start=(t == 0), stop=(t == n_tiles - 1))
                for h in range(4):
                    nc.tensor.matmul(zap[h], ex[:, j, 128 * h:128 * h + 128],
                                     vals[:, t, :], **st)

        sizes = [GT] * (n_tiles // GT)
        pending = []
        t0 = 0
        for gi, gn in enumerate(sizes):
            sp = psc.tile([128, GT, QC], F32, tag="sp")
            for j in range(gn):
                nc.tensor.matmul(sp[:, j, :], keysT[:, t0 + j, :], qs,
                                 start=True, stop=True)
            if len(pending) >= 3:
                emit_pv(*pending.pop(0))
            ex = pex.tile([128, GT, QC], BF16, tag="ex")
            if gi in dve_set:
                nc.vector.tensor_scalar(
                    ex[:, 0:gn, :].bitcast(I16), sp[:, 0:gn, :],
                    EXPA, EXPB, op0=ALU.mult, op1=ALU.add)
            else:
                nc.scalar.activation(ex[:, 0:gn, :], sp[:, 0:gn, :], AF.Exp,
                                     scale=SCALE)
            pending.append((ex, t0, gn))
            t0 += gn
            if interleave:
                interleave.pop(0)()
        for p in pending:
            emit_pv(*p)
        return zap

    def normalize_c(zs, qb):
        # compressed: zcomp[:, qt, :] = z' / S
        zrs, invs = [], []
        for h, zp in enumerate(zs):
            zr = psmall.tile([128, 129], F32, tag="zr", name=f"zr{h}")
            nc.scalar.copy(zr[:], zp)
            zrs.append(zr)
        for h in range(4):
            inv = psmall.tile([128, 1], F32, tag="inv", name=f"inv{h}")
            nc.vector.reciprocal(inv[:], zrs[h][:, 128:129])
            invs.append(inv)
        for h in range(4):
            nc.vector.tensor_scalar_mul(zcomp[:, 4 * qb + h, :],
                                        zrs[h][:, 0:128], invs[h][:])

    def normalize_t(zs, qb):
        # teacher: accq[:, qt] = sum_d (z'/S - zcomp)^2
        zrs, invs, ds, d2s = [], [], [], []
        for h, zp in enumerate(zs):
            zr = psmall.tile([128, 129], F32, tag="ztr", name=f"ztr{h}")
            nc.scalar.copy(zr[:], zp)
            zrs.append(zr)
        for h in range(4):
            inv = psmall.tile([128, 1], F32, tag="inv", name=f"inv{h}")
            nc.vector.reciprocal(inv[:], zrs[h][:, 128:129])
            invs.append(inv)
        for h in range(4):
            zn = psmall.tile([128, 128], F32, tag="zn", name=f"zn{h}")
            nc.vector.tensor_scalar_mul(zn[:], zrs[h][:, 0:128], invs[h][:])
            d = psmall.tile([128, 128], F32, tag="d", name=f"d{h}")
            nc.vector.tensor_sub(d[:], zn[:], zcomp[:, 4 * qb + h, :])
            ds.append(d)
        for h in range(4):
            d2 = psmall.tile([128, 128], F32, tag="d2", name=f"d2{h}")
            nc.scalar.square(d2[:], ds[h][:])
            d2s.append(d2)
        for h in range(4):
            nc.vector.reduce_sum(out=accq[:, 4 * qb + h:4 * qb + h + 1],
                                 in_=d2s[h][:], axis=mybir.AxisListType.X)

    # ---- main schedule ----
    stage_transposed(qh, qT, 8, "stq")
    stage_transposed(ckh, ckT, 8, "stck")
    stage_cv()

    nop = lambda: None

    # c0: start the k stream (chunks 0-4)
    il = [lambda g=g: k_chunk(g) for g in range(5)] + [nop] * 3
    zs = attend(ckT, cvb, NCT, 0, DVE_C0, il)
    normalize_c(zs, 0)

    # t0: v chunks early (PV consumes them almost immediately), rest of k
    il = []
    ks = list(range(5, 16))
    for g in range(16):
        il.append(lambda g=g: v_chunk(g))
        if ks:
            il.append(lambda g=ks.pop(0): k_chunk(g))
    il += [nop] * (64 - len(il))
    zs = attend(kT, vb, NT, 0, DVE_T0, il)
    normalize_t(zs, 0)

    zs = attend(ckT, cvb, NCT, 1, DVE_C1, None)
    normalize_c(zs, 1)

    zs = attend(kT, vb, NT, 1, DVE_T1, None)
    normalize_t(zs, 1)

    nc.sync.dma_start(out=out_dram[:], in_=accq[:])

    for p in reversed(ctxs):
        p.__exit__(None, None, None)


_NC_CACHE = None


def build_nc():
    global _NC_CACHE
    if _NC_CACHE is not None:
        return _NC_CACHE
    nc = bacc.Bacc()
    qh = nc.declare_dram_parameter("queries", [Q, D], F32, isOutput=False)
    kh = nc.declare_dram_parameter("keys", [N, D], F32, isOutput=False)
    vh = nc.declare_dram_parameter("values", [N, D], F32, isOutput=False)
    ckh = nc.declare_dram_parameter("c_keys", [NC, D], F32, isOutput=False)
    cvh = nc.declare_dram_parameter("c_values", [NC, D], F32, isOutput=False)
    out = nc.declare_dram_parameter("loss_sums", [128, Q // 128], F32, isOutput=True)
    with tile.TileContext(nc) as tc:
        _emit(nc, tc, qh, kh, vh, ckh, cvh, out)
    nc.compile()
    _NC_CACHE = nc
    return nc


def make_in_maps(queries, keys, values, c_keys, c_values):
    in_maps = []
    for h in range(N_CORES):
        in_maps.append({
            "queries": np.ascontiguousarray(queries[0, h], dtype=np.float32),
            "keys": np.ascontiguousarray(keys[0, h], dtype=np.float32),
            "values": np.ascontiguousarray(values[0, h], dtype=np.float32),
            "c_keys": np.ascontiguousarray(c_keys[0, h], dtype=np.float32),
            "c_values": np.ascontiguousarray(c_values[0, h], dtype=np.float32),
        })
    return in_maps


def run_cores(in_maps, trace=False, **kw):
    nc = build_nc()
    return run_bass_kernel_spmd(nc, in_maps, list(range(N_CORES)),
                                trace=trace, **kw)


def kernel(queries, keys, values, c_keys, c_values):
    res = run_cores(make_in_maps(queries, keys, values, c_keys, c_values))
    total = sum(float(r["loss_sums"].astype(np.float64).sum())
                for r in res.results)
    loss = total / float(B * H * Q * D)
    return np.asarray(loss, dtype=np.float32)


# revision 23
# speedup vs baseline: 1.0488x; 1.0488x over previous
"""Distillation-trainer loss kernel for Trainium2 (8 NeuronCores).

Computes  loss = mean((attn(q,k,v) - attn(q,ck,cv))**2)  for
q:[1,8,1024,128], k/v:[1,8,8192,128], ck/cv:[1,8,1024,128] fp32.

Sharding: one kv-head per core (h axis, 8 heads / 8 cores). Each core
computes its head's squared-error partial sums; the host adds the 8
partials and divides by the element count (the "all-reduce" of the
scalar loss).

Per-core pipeline (head h), blocks ordered c0, t0, c1, t1 where c =
compressed attention, t = teacher attention, each over a 512-wide
q-block:
  - scores: one 512-wide matmul per 128-row n-tile (kT stationary,
    qT moving, PSUM fp32).
  - exp is split across two engines to beat the ScalarE throughput
    wall (~1.2 ns/elem): ACT groups use the Exp table; DVE groups use
    a one-instruction Schraudolph exp - t = s*A + B converted to int16,
    whose bits ARE bf16(exp(s*scale)) because bf16's exponent/mantissa
    are linear in log2. Probs land in bf16 either way (loss rel-err
    ~1e-4, tolerance 2e-2).
  - PV: ex chunk [128n x 128q] stationary, moving v-tile [128, 129]
    with a ones column appended, accumulating [z' | S] per q-quarter
    in four PSUM banks across n-tiles.
  - normalize on DVE (reciprocal of S), MSE via fused
    tensor_tensor_reduce into per-partition partials.
  - k/v HBM streams (8.4 MB) are interleaved into c0/t0 so DMA hides
    under compute; kT PSUM->SBUF copy-outs run on GpSimd to unload DVE.
"""

import numpy as np

import concourse.bass as bass
import concourse.mybir as mybir
import concourse.tile as tile
from concourse import bacc
from concourse.masks import make_identity
from concourse.bass_utils import run_bass_kernel_spmd

F32 = mybir.dt.float32
BF16 = mybir.dt.bfloat16
I16 = mybir.dt.int16
AF = mybir.ActivationFunctionType
ALU = mybir.AluOpType

B, H, Q, N, NC, D = 1, 8, 1024, 8192, 1024, 128
N_CORES = 8
SCALE = 1.0 / float(np.sqrt(D))

QC = 512                   # q-block width
N_QB = Q // QC             # 2
GT = 1                     # n-tiles per scores/exp group
NT = N // 128              # 64 teacher n-tiles
NCT = NC // 128            # 8 compressed n-tiles

# Schraudolph exp constants: int16(raw_score * EXPA + EXPB) bits == bf16
# of exp(raw_score * SCALE).  EXPA folds the softmax scale and log2(e)
# into the bf16 exponent/mantissa field width (128 = 2^7 mantissa bits).
EXPC = 5.8                 # mantissa bias correction (minimizes mean err)
EXPA = float(SCALE * np.log2(np.e) * 128.0)
EXPB = float(127.0 * 128.0 - EXPC)

# which exp groups run on DVE (rest on ACT); tuned so ACT/DVE/PE balance
DVE_C0 = frozenset((1, 3, 5, 7))
DVE_C1 = frozenset((1, 3, 5, 7))
DVE_T0 = frozenset(i for i in range(64) if i % 16 in (1, 4, 7, 10, 13))  # 20 of 64
DVE_T1 = frozenset(i for i in range(64) if i % 2 == 1)      # 32 of 64
GP_KCOPY = False           # GPSIMD cannot read PSUM (walrus birverifier)
USE_TTR = False            # tensor_tensor_reduce hangs real TRN2 (sim-only)


def _emit(nc: bass.Bass, tc: tile.TileContext, qh, kh, vh, ckh, cvh, out_dram):
    ctxs = []

    def pool(**kw):
        p = tc.tile_pool(**kw)
        ctxs.append(p)
        return p.__enter__()

    pconst = pool(name="pconst", bufs=1)
    pkst = pool(name="pkst", bufs=3)
    pvst = pool(name="pvst", bufs=3)
    pex = pool(name="pex", bufs=4)
    psmall = pool(name="psmall", bufs=4)
    psc = pool(name="psc", bufs=4, space="PSUM")
    pz = pool(name="pz", bufs=1, space="PSUM")

    # ---- persistent SBUF tensors ----
    ident = pconst.tile([128, 128], BF16, tag="ident")
    make_identity(nc, ident[:])

    kT = pconst.tile([128, NT, 128], BF16, tag="kT")        # [d, t, n]
    vb = pconst.tile([128, NT, 129], BF16, tag="vb")        # [n, t, d+1]
    qT = pconst.tile([128, 8, 128], BF16, tag="qT")         # [d, qt, q]
    ckT = pconst.tile([128, NCT, 128], BF16, tag="ckT")
    cvb = pconst.tile([128, NCT, 129], BF16, tag="cvb")
    zcomp = pconst.tile([128, Q // 128, 128], F32, tag="zcomp")  # [q, qt, d]
    accq = pconst.tile([128, Q // 128], F32, tag="accq")

    nc.gpsimd.memset(vb[:, :, 128:129], 1.0)
    nc.gpsimd.memset(cvb[:, :, 128:129], 1.0)

    # Warm the ACT exp table while prep DMAs run (~2.7us table load).
    warm = psmall.tile([128, 1], F32, tag="warm")
    nc.gpsimd.memset(warm[:], 0.0)
    warm2 = psmall.tile([128, 1], F32, tag="warm2")
    nc.scalar.activation(warm2[:], warm[:], AF.Exp)

    # ---- staging helpers ----
    def stage_transposed(src, dst, n_tiles, tag):
        # src [n, 128] fp32 -> dst [128, t, n] bf16 via cast + PE transpose
        stg = pkst.tile([128, n_tiles, 128], F32, tag=tag)
        nc.sync.dma_start(out=stg[:],
                          in_=src[:, :].rearrange("(i p) d -> p i d", p=128))
        sb = pkst.tile([128, n_tiles, 128], BF16, tag=tag + "b")
        nc.vector.tensor_copy(sb[:], stg[:])
        for g in range(n_tiles // 4):
            tp = psc.tile([128, 4, 128], BF16, tag="sp")
            for j in range(4):
                nc.tensor.transpose(tp[:, j, :], sb[:, 4 * g + j, :], ident[:])
            nc.vector.tensor_copy(dst[:, 4 * g:4 * g + 4, :], tp[:])

    def k_chunk(g):
        # 512 rows of k -> kT[:, 4g:4g+4, :]
        stg = pkst.tile([128, 4, 128], F32, tag="kstg")
        ap = kh[g * 512:(g + 1) * 512, :].rearrange("(i p) d -> p i d", p=128)
        nc.sync.dma_start(out=stg[:], in_=ap)
        kb = pkst.tile([128, 4, 128], BF16, tag="kb")
        nc.vector.tensor_copy(kb[:], stg[:])
        tp = psc.tile([128, 4, 128], BF16, tag="sp")
        for j in range(4):
            nc.tensor.transpose(tp[:, j, :], kb[:, j, :], ident[:])
        eng = nc.gpsimd if GP_KCOPY else nc.vector
        eng.tensor_copy(kT[:, 4 * g:4 * g + 4, :], tp[:])

    def v_chunk(g):
        stg = pvst.tile([128, 4, 128], F32, tag="vstg")
        ap = vh[g * 512:(g + 1) * 512, :].rearrange("(i p) d -> p i d", p=128)
        nc.sync.dma_start(out=stg[:], in_=ap)
        nc.vector.tensor_copy(vb[:, 4 * g:4 * g + 4, 0:128], stg[:])

    def stage_cv():
        stg = pvst.tile([128, NCT, 128], F32, tag="cvstg")
        nc.sync.dma_start(out=stg[:],
                          in_=cvh[:, :].rearrange("(i p) d -> p i d", p=128))
        nc.vector.tensor_copy(cvb[:, :, 0:128], stg[:])

    # ---- attention for one 512-wide q-block ----
    def attend(keysT, vals, n_tiles, qb, dve_set, interleave):
        """Returns (zA, zB): zA [128,3,129] holds q-quarters 0-2, zB [128,129]
        holds quarter 3 -- each column block is [z' | S].  The four PV
        accumulation chains share PSUM banks; per-element has_written bits
        keep them independent (skip_group_check)."""
        zap = [pz.tile([128, 129], F32, tag=f"z{h}", name=f"z{h}")[:]
               for h in range(4)]
        qs = qT[:, 4 * qb:4 * qb + 4, :]

        def emit_pv(ex, t0, gn):
            for j in range(gn):
                t = t0 + j
                st = dict(start=(t == 0), stop=(t == n_tiles - 1))
                for h in range(4):
                    nc.tensor.matmul(zap[h], ex[:, j, 128 * h:128 * h + 128],
                                     vals[:, t, :], **st)

        sizes = [GT] * (n_tiles // GT)
        pending = []
        t0 = 0
        for gi, gn in enumerate(sizes):
            sp = psc.tile([128, GT, QC], F32, tag="sp")
            for j in range(gn):
                nc.tensor.matmul(sp[:, j, :], keysT[:, t0 + j, :], qs,
                                 start=True, stop=True)
            if len(pending) >= 3:
                emit_pv(*pending.pop(0))
            ex = pex.tile([128, GT, QC], BF16, tag="ex")
            if gi in dve_set:
                nc.vector.tensor_scalar(
                    ex[:, 0:gn, :].bitcast(I16), sp[:, 0:gn, :],
                    EXPA, EXPB, op0=ALU.mult, op1=ALU.add)
            else:
                nc.scalar.activation(ex[:, 0:gn, :], sp[:, 0:gn, :], AF.Exp,
                                     scale=SCALE)
            pending.append((ex, t0, gn))
            t0 += gn
            if interleave:
                interleave.pop(0)()
        for p in pending:
            emit_pv(*p)
        return zap

    def normalize_c(zs, qb):
        # compressed: zcomp[:, qt, :] = z' / S
        zrs, invs = [], []
        for h, zp in enumerate(zs):
            zr = psmall.tile([128, 129], F32, tag="zr", name=f"zr{h}")
            nc.scalar.copy(zr[:], zp)
            zrs.append(zr)
        for h in range(4):
            inv = psmall.tile([128, 1], F32, tag="inv", name=f"inv{h}")
            nc.vector.reciprocal(inv[:], zrs[h][:, 128:129])
            invs.append(inv)
        for h in range(4):
            nc.vector.tensor_scalar_mul(zcomp[:, 4 * qb + h, :],
                                        zrs[h][:, 0:128], invs[h][:])

    def normalize_t(zs, qb):
        # teacher: accq[:, qt] = sum_d (z'/S - zcomp)^2
        zrs, invs, ds, d2s = [], [], [], []
        for h, zp in enumerate(zs):
            zr = psmall.tile([128, 129], F32, tag="ztr", name=f"ztr{h}")
            nc.scalar.copy(zr[:], zp)
            zrs.append(zr)
        for h in range(4):
            inv = psmall.tile([128, 1], F32, tag="inv", name=f"inv{h}")
            nc.vector.reciprocal(inv[:], zrs[h][:, 128:129])
            invs.append(inv)
        for h in range(4):
            zn = psmall.tile([128, 128], F32, tag="zn", name=f"zn{h}")
            nc.vector.tensor_scalar_mul(zn[:], zrs[h][:, 0:128], invs[h][:])
            d = psmall.tile([128, 128], F32, tag="d", name=f"d{h}")
            nc.vector.tensor_sub(d[:], zn[:], zcomp[:, 4 * qb + h, :])
            ds.append(d)
        for h in range(4):
            d2 = psmall.tile([128, 128], F32, tag="d2", name=f"d2{h}")
            nc.scalar.square(d2[:], ds[h][:])
            d2s.append(d2)
        for h in range(4):
            nc.vector.reduce_sum(out=accq[:, 4 * qb + h:4 * qb + h + 1],
                                 in_=d2s[h][:], axis=mybir.AxisListType.X)

    # ---- main schedule ----
    stage_transposed(qh, qT, 8, "stq")
    stage_transposed(ckh, ckT, 8, "stck")
    stage_cv()

    nop = lambda: None

    # c0: start the k stream (chunks 0-4)
    il = [lambda g=g: k_chunk(g) for g in range(5)] + [nop] * 3
    zs = attend(ckT, cvb, NCT, 0, DVE_C0, il)
    normalize_c(zs, 0)

    # t0: v chunks early (PV consumes them almost immediately), rest of k
    il = []
    ks = list(range(5, 16))
    for g in range(16):
        il.append(lambda g=g: v_chunk(g))
        if ks:
            il.append(lambda g=ks.pop(0): k_chunk(g))
    il += [nop] * (64 - len(il))
    zs = attend(kT, vb, NT, 0, DVE_T0, il)
    normalize_t(zs, 0)

    zs = attend(ckT, cvb, NCT, 1, DVE_C1, None)
    normalize_c(zs, 1)

    zs = attend(kT, vb, NT, 1, DVE_T1, None)
    normalize_t(zs, 1)

    nc.sync.dma_start(out=out_dram[:], in_=accq[:])

    for p in reversed(ctxs):
        p.__exit__(None, None, None)


_NC_CACHE = None


def build_nc():
    global _NC_CACHE
    if _NC_CACHE is not None:
        return _NC_CACHE
    nc = bacc.Bacc()
    qh = nc.declare_dram_parameter("queries", [Q, D], F32, isOutput=False)
    kh = nc.declare_dram_parameter("keys", [N, D], F32, isOutput=False)
    vh = nc.declare_dram_parameter("values", [N, D], F32, isOutput=False)
    ckh = nc.declare_dram_parameter("c_keys", [NC, D], F32, isOutput=False)
    cvh = nc.declare_dram_parameter("c_values", [NC, D], F32, isOutput=False)
    out = nc.declare_dram_parameter("loss_sums", [128, Q // 128], F32, isOutput=True)
    with tile.TileContext(nc) as tc:
        _emit(nc, tc, qh, kh, vh, ckh, cvh, out)
    nc.compile()
    _NC_CACHE = nc
    return nc


def make_in_maps(queries, keys, values, c_keys, c_values):
    in_maps = []
    for h in range(N_CORES):
        in_maps.append({
            "queries": np.ascontiguousarray(queries[0, h], dtype=np.float32),
            "keys": np.ascontiguousarray(keys[0, h], dtype=np.float32),
            "values": np.ascontiguousarray(values[0, h], dtype=np.float32),
            "c_keys": np.ascontiguousarray(c_keys[0, h], dtype=np.float32),
            "c_values": np.ascontiguousarray(c_values[0, h], dtype=np.float32),
        })
    return in_maps


def run_cores(in_maps, trace=False, **kw):
    nc = build_nc()
    return run_bass_kernel_spmd(nc, in_maps, list(range(N_CORES)),
                                trace=trace, **kw)


def kernel(queries, keys, values, c_keys, c_values):
    res = run_cores(make_in_maps(queries, keys, values, c_keys, c_values))
    total = sum(float(r["loss_sums"].astype(np.float64).sum())
                for r in res.results)
    loss = total / float(B * H * Q * D)
    return np.asarray(loss, dtype=np.float32)
